# revision 4
# baseline (speedup 1.0000x reference)
"""ConstituencyAwareAttention Trainium2 kernel (v2).

Strategy: pure data parallelism -- B=8 batch elements across 8 NeuronCores,
one full attention problem per core (S=1024, H=1024, nh=16, hd=64).

v2 restructure (vs v1 baseline at ~331us):
  * Chunked, prioritized input DMA: X chunks + first Q/K weight slices land
    first, so the bootstrap Q/K projection starts at ~10us instead of 40us.
    Remaining weight slices stream in under compute.
  * Software-pipelined pair slots: slot i emits scores(i)+exp(i) interleaved
    per-kt with proj(i+1) bursts and AV(i-1)+transpose(i-1) bursts, so the
    PE never drains while the scalar engine (exp, the secondary bottleneck
    at ~142us total) is continuously fed.
  * All matmul accumulation runs in single-bank bursts (8 chained MMs) so
    PSUM fits in exactly 8 banks: scores 4 (2 halves x [128,1024]) +
    proj ring 2 + work ring 2 (shared by V-proj / AV-ctx / PE-transposes).
  * exp is scalar-engine-only; penalty-fix multiplies moved to gpsimd;
    PSUM evacuations on vector -- keeps the exp stream stall-free.
  * Output shipped per head-pair from a small rotating buffer.
"""

import math
import sys

if "/opt/trn_rl_repo" not in sys.path:
    sys.path.insert(0, "/opt/trn_rl_repo")

import numpy as np

import concourse.bacc as bacc
import concourse.tile as tile
from concourse import mybir
from concourse.bass_utils import run_bass_kernel_spmd
from concourse.masks import make_identity

F16 = mybir.dt.float16
F32 = mybir.dt.float32

B, S, H = 8, 1024, 1024
NH, HD = 16, 64
P = 128
SO = S // P   # 8 S-chunks
KO = H // P   # 8 contraction chunks
NP = NH // 2  # 8 head pairs
PEN = 0.5
FIX = float(math.exp(PEN))
SCALE = 1.0 / math.sqrt(HD)

_programs = {}


def _build_program(with_bv: bool):
    nc = bacc.Bacc("TRN2", target_bir_lowering=False, debug=False)

    xt = nc.dram_tensor("xt", [H, S], F32, kind="ExternalInput").ap()
    wq = nc.dram_tensor("wq", [H, H], F32, kind="ExternalInput").ap()
    wk = nc.dram_tensor("wk", [H, H], F32, kind="ExternalInput").ap()
    wv = nc.dram_tensor("wv", [H, H], F32, kind="ExternalInput").ap()
    bq = nc.dram_tensor("bq", [H], F32, kind="ExternalInput").ap()
    bk = nc.dram_tensor("bk", [H], F32, kind="ExternalInput").ap()
    bv = nc.dram_tensor("bv", [H], F32, kind="ExternalInput").ap()
    out = nc.dram_tensor("out", [S, H], F32, kind="ExternalOutput").ap()

    Exp = mybir.ActivationFunctionType.Exp

    xt_r = xt.rearrange("(ho hp) s -> hp ho s", hp=P)      # [128, KO, S]
    wq_r = wq.rearrange("(kp_o kp) n -> kp kp_o n", kp=P)  # [128, KO, H]
    wk_r = wk.rearrange("(kp_o kp) n -> kp kp_o n", kp=P)
    wv_r = wv.rearrange("(kp_o kp) n -> kp kp_o n", kp=P)
    out_r = out.rearrange("(o p) n -> p o n", p=P)         # [128, SO, H]

    with tile.TileContext(nc) as tc:
        with (
            tc.tile_pool(name="persist", bufs=1) as persist,
            tc.tile_pool(name="stage", bufs=1) as stage,
            tc.tile_pool(name="probs", bufs=1) as probs,
            tc.tile_pool(name="score_ps", bufs=1, space="PSUM") as score_ps,
            tc.tile_pool(name="proj_ps", bufs=2, space="PSUM") as proj_ps,
            tc.tile_pool(name="work_ps", bufs=2, space="PSUM") as work_ps,
        ):
            # ---------------- persistent SBUF ----------------
            XT = persist.tile([P, KO, S], F16, name="XT")
            # Q^T/K^T live as 3-deep per-pair rings (each pair's slice is
            # produced one slot ahead of its scores)
            QT3 = persist.tile([P, 3, S], F16, name="QT3")
            KT3 = persist.tile([P, 3, S], F16, name="KT3")
            VA = persist.tile([P, SO, NH * (HD + 1)], F16, name="VA")
            wqh = persist.tile([P, KO, H], F16, name="wqh")
            wkh = persist.tile([P, KO, H], F16, name="wkh")
            wvh = persist.tile([P, KO, H], F16, name="wvh")
            ident = persist.tile([P, P], F32, name="ident")
            nbias = persist.tile([P, 1], F32, name="nbias")
            bq_s = persist.tile([P, KO], F32, name="bq_s")
            bk_s = persist.tile([P, KO], F32, name="bk_s")

            make_identity(nc, ident[:])
            nc.vector.memset(nbias[:], -PEN)
            # ones columns of V_aug only; V block copies fill the rest
            VA_v = VA[:].rearrange("p s (h c) -> p s h c", c=HD + 1)
            nc.vector.memset(VA_v[:, :, :, HD : HD + 1], 1.0)

            # ---------------- DMA: chunked, prioritized ----------------
            # Only 3 hw DMA queues: sync (SP), scalar (Activation), gpsimd.
            bq32 = stage.tile([P, KO], F32, name="bq32")
            bk32 = stage.tile([P, KO], F32, name="bk32")
            nc.sync.dma_start(out=bq32[:], in_=bq.rearrange("(o p) -> p o", p=P))
            nc.sync.dma_start(out=bk32[:], in_=bk.rearrange("(o p) -> p o", p=P))
            nc.vector.tensor_scalar_mul(bq_s[:], bq32[:], SCALE)
            nc.vector.tensor_copy(bk_s[:], bk32[:])

            # fp32 staging rings
            x32 = [
                stage.tile([P, S], F32, name=f"x32_{k}", tag="x32", bufs=2)
                for k in range(KO)
            ]
            wq32 = [None] * NP
            wk32 = [None] * NP
            wv32 = [None] * KO
            for mo in range(NP):
                wq32[mo] = stage.tile([P, KO, P], F32, name=f"wq32_{mo}", tag="w32", bufs=3)
                wk32[mo] = stage.tile([P, KO, P], F32, name=f"wk32_{mo}", tag="w32", bufs=3)
            for kh in range(KO):
                wv32[kh] = stage.tile([P, H], F32, name=f"wv32_{kh}", tag="wv32", bufs=2)

            # first q/k slices (bootstrap + slot-0 proj) ahead of X on sync;
            # X split across all 3 queues; Wv next; remaining q/k slices in
            # need order round-robin.
            nc.sync.dma_start(out=wq32[0][:], in_=wq_r[:, :, 0:P])
            nc.sync.dma_start(out=wk32[0][:], in_=wk_r[:, :, 0:P])
            for kh, eng in [(1, nc.scalar), (4, nc.scalar), (7, nc.scalar),
                            (2, nc.gpsimd), (5, nc.gpsimd),
                            (0, nc.sync), (3, nc.sync), (6, nc.sync)]:
                eng.dma_start(out=x32[kh][:], in_=xt_r[:, kh, :])
            nc.scalar.dma_start(out=wq32[1][:], in_=wq_r[:, :, P : 2 * P])
            nc.gpsimd.dma_start(out=wk32[1][:], in_=wk_r[:, :, P : 2 * P])
            for kh in range(KO):
                eng = nc.scalar if kh % 2 == 0 else nc.gpsimd
                eng.dma_start(out=wv32[kh][:], in_=wv_r[:, kh, :])
            rr = [nc.sync, nc.scalar, nc.gpsimd]
            idx = 0
            for mo in range(2, NP):
                rr[idx % 3].dma_start(out=wq32[mo][:], in_=wq_r[:, :, mo * P : (mo + 1) * P])
                idx += 1
                rr[idx % 3].dma_start(out=wk32[mo][:], in_=wk_r[:, :, mo * P : (mo + 1) * P])
                idx += 1

            wqh_v = wqh.rearrange("p o (m c) -> p o m c", c=P)
            wkh_v = wkh.rearrange("p o (m c) -> p o m c", c=P)

            # X casts on vector (gate everything)
            for kh in range(KO):
                nc.vector.tensor_copy(XT[:, kh, :], x32[kh][:])
            # first two q/k slice casts on vector (fast, before the exp storm)
            for mo in range(2):
                nc.vector.tensor_copy(wqh_v[:, :, mo, :], wq32[mo][:])
                nc.vector.tensor_copy(wkh_v[:, :, mo, :], wk32[mo][:])
            # Wv casts split vector/gpsimd (gate V bursts in slot 0)
            for kh in range(KO):
                eng = nc.vector if kh % 2 == 0 else nc.gpsimd
                eng.tensor_copy(wvh[:, kh, :], wv32[kh][:])

            # probs rings: 2 pairs in flight (scored / being consumed by AV)
            prT = [
                [
                    probs.tile([P, KO, S], F16, name=f"prT_{h}_{r}")
                    for r in range(2)
                ]
                for h in range(2)
            ]
            ctxt_sb = [
                probs.tile([HD + 1, 512], F32, name=f"ctxt_sb{r}")
                for r in range(4)
            ]
            inv_sb = [
                probs.tile([P, 4], F32, name=f"inv{r}")
                for r in range(4)
            ]
            out_pair = [
                probs.tile([P, SO, P], F32, name=f"out_pair{r}")
                for r in range(2)
            ]

            # score psum: one [128, S] tile per half, reused across kt
            pst = [
                score_ps.tile([P, S], F32, name=f"pst{h}", tag=f"pst{h}")
                for h in range(2)
            ]

            # ---------------- emission helpers ----------------
            def cast_w_slice(mo):
                """fp16-cast q/k slice mo on gpsimd (JIT, keeps vector free)."""
                nc.gpsimd.tensor_copy(wqh_v[:, :, mo, :], wq32[mo][:])
                nc.gpsimd.tensor_copy(wkh_v[:, :, mo, :], wk32[mo][:])

            def proj_burst(mo, which, sc):
                """One Q or K projection burst: 8 chained MMs -> 1 psum bank,
                evacuated to the QT/KT ring with scale+bias fused."""
                wsb = wqh if which == "q" else wkh
                r = mo % 3
                ps = proj_ps.tile([P, 512], F32, name="ps", tag="proj")
                for kh in range(KO):
                    nc.tensor.matmul(
                        ps[:],
                        wsb[:, kh, mo * P : (mo + 1) * P],
                        XT[:, kh, sc * 512 : (sc + 1) * 512],
                        start=(kh == 0),
                        stop=(kh == KO - 1),
                    )
                if which == "q":
                    nc.vector.tensor_scalar(
                        QT3[:, r, sc * 512 : (sc + 1) * 512], ps[:],
                        SCALE, bq_s[:, mo : mo + 1],
                        mybir.AluOpType.mult, mybir.AluOpType.add,
                    )
                else:
                    nc.vector.tensor_scalar_add(
                        KT3[:, r, sc * 512 : (sc + 1) * 512], ps[:],
                        bk_s[:, mo : mo + 1],
                    )

            def v_burst(so, ncol):
                """V projection burst: 8 chained MMs -> work ring bank, then
                strided copy into V_aug (leaving the ones columns alone)."""
                ps = work_ps.tile([P, 512], F32, name="vps", tag="work")
                for kh in range(KO):
                    nc.tensor.matmul(
                        ps[:],
                        XT[:, kh, so * P : (so + 1) * P],
                        wvh[:, kh, ncol * 512 : (ncol + 1) * 512],
                        start=(kh == 0),
                        stop=(kh == KO - 1),
                    )
                va_v = VA[:, so, :].rearrange("p (h c) -> p h c", c=HD + 1)
                nc.vector.tensor_copy(
                    va_v[:, ncol * 8 : (ncol + 1) * 8, 0:HD],
                    ps[:].rearrange("p (h c) -> p h c", c=HD),
                )

            def score_kt(i, kt, ring):
                """scores + exp for (pair i, k-chunk kt); 4 row-packed MMs,
                2 scalar exps, gpsimd penalty-fix."""
                r = i % 3
                for half in range(2):
                    lo = half * 64
                    for qc in range(2):
                        nc.tensor.matmul(
                            pst[half][:, qc * 512 : (qc + 1) * 512],
                            KT3[lo : lo + 64, r, kt * P : (kt + 1) * P],
                            QT3[lo : lo + 64, r, qc * 512 : (qc + 1) * 512],
                            start=True,
                            stop=True,
                            tile_position=(lo, 0),
                        )
                for half in range(2):
                    dst = prT[half][ring]
                    nc.scalar.activation(
                        dst[:, kt, :], pst[half][:], Exp, bias=nbias[:]
                    )
                    nc.gpsimd.tensor_scalar_mul(
                        dst[0:64, kt, kt * P : kt * P + 64],
                        dst[0:64, kt, kt * P : kt * P + 64],
                        FIX,
                    )
                    nc.gpsimd.tensor_scalar_mul(
                        dst[64:128, kt, kt * P + 64 : (kt + 1) * P],
                        dst[64:128, kt, kt * P + 64 : (kt + 1) * P],
                        FIX,
                    )

            def av_burst(i, half, qc, ring, slot4):
                """AV for (pair i, head-half, q-chunk qc): 8 chained MMs into
                a work-ring bank (65 rows: 64 ctx dims + denominator), then
                fp32 evacuation to SBUF."""
                h = 2 * i + half
                ctx = work_ps.tile([P, 512], F32, name="ctx", tag="work")
                for kt in range(KO):
                    nc.tensor.matmul(
                        ctx[0 : HD + 1, :],
                        VA[:, kt, h * (HD + 1) : (h + 1) * (HD + 1)],
                        prT[half][ring][:, kt, qc * 512 : (qc + 1) * 512],
                        start=(kt == 0),
                        stop=(kt == KO - 1),
                    )
                nc.vector.tensor_copy(ctxt_sb[slot4][:], ctx[0 : HD + 1, :])

            def tp_burst(half, qc, slot4, opr):
                """Transpose ctx^T back to [q, d] + normalize into out_pair."""
                tp = work_ps.tile([P, 512], F32, name="tp", tag="work")
                tpv = tp[:, 0 : 4 * (HD + 1)].rearrange(
                    "p (c e) -> p c e", e=HD + 1
                )
                for c4 in range(4):
                    nc.tensor.transpose(
                        tpv[:, c4, :],
                        ctxt_sb[slot4][:, c4 * P : (c4 + 1) * P],
                        ident[0 : HD + 1, 0 : HD + 1],
                    )
                nc.vector.reciprocal(inv_sb[slot4][:], tpv[:, :, HD])
                for c4 in range(4):
                    so = qc * 4 + c4
                    nc.vector.tensor_scalar_mul(
                        out_pair[opr][:, so, half * HD : (half + 1) * HD],
                        tpv[:, c4, 0:HD],
                        inv_sb[slot4][:, c4 : c4 + 1],
                    )

            def ship_pair(i, opr):
                nc.sync.dma_start(
                    out=out_r[:, :, i * P : (i + 1) * P], in_=out_pair[opr][:]
                )

            # ---------------- bootstrap: proj(0) ----------------
            for sc in range(2):
                proj_burst(0, "q", sc)
            for sc in range(2):
                proj_burst(0, "k", sc)

            # ---------------- pair slots ----------------
            # slot i: scores(i) + exp(i) | proj(i+1) | slot0: V proj
            #         | i>=1: AV(i-1) + tp(i-1) + ship(i-1)
            av_seq = [(0, 0), (0, 1), (1, 0), (1, 1)]
            for i in range(NP):
                ring = i % 2
                pring = (i - 1) % 2
                opr = (i - 1) % 2

                for kt in range(KO):
                    score_kt(i, kt, ring)

                    # projection burst piece for pair i+1 (one per 2 steps)
                    if i + 1 < NP:
                        if kt == 0:
                            proj_burst(i + 1, "q", 0)
                        elif kt == 2:
                            proj_burst(i + 1, "q", 1)
                        elif kt == 4:
                            proj_burst(i + 1, "k", 0)
                        elif kt == 6:
                            proj_burst(i + 1, "k", 1)
                        elif kt == 7 and i + 2 < NP:
                            cast_w_slice(i + 2)

                    if i == 0:
                        # V projection fills the AV slots of slot 0
                        if kt < 4:
                            v_burst(2 * kt, 0)
                            v_burst(2 * kt, 1)
                        else:
                            v_burst(2 * (kt - 4) + 1, 0)
                            v_burst(2 * (kt - 4) + 1, 1)
                    else:
                        # AV(i-1) bursts on even steps, tp(i-1) on odd steps
                        if kt % 2 == 0:
                            half, qc = av_seq[kt // 2]
                            av_burst(i - 1, half, qc, pring, kt // 2)
                        elif kt >= 3:
                            half, qc = av_seq[(kt - 3) // 2]
                            tp_burst(half, qc, (kt - 3) // 2, opr)

                if i == 0 and with_bv:
                    # out += bv exactly (softmax rows sum to 1): broadcast bv
                    # across partitions via PE, add into V_aug.
                    ones1 = persist.tile([1, P], F16, name="ones1")
                    nc.vector.memset(ones1[:], 1.0)
                    bv1 = persist.tile([1, H], F16, name="bv1")
                    bv1_32 = persist.tile([1, H], F32, name="bv1_32")
                    nc.sync.dma_start(out=bv1_32[:], in_=bv[None, :])
                    nc.vector.tensor_copy(bv1[:], bv1_32[:])
                    bvb = persist.tile([P, NH * (HD + 1)], F16, name="bvb")
                    nc.vector.memset(bvb[:], 0.0)
                    bvb_v = bvb.rearrange("p (h c) -> p h c", c=HD + 1)
                    for ncol in range(2):
                        psb = proj_ps.tile([P, 512], F32, name="psb", tag="proj")
                        nc.tensor.matmul(
                            psb[:], ones1[:], bv1[:, ncol * 512 : (ncol + 1) * 512],
                            start=True, stop=True,
                        )
                        nc.vector.tensor_copy(
                            bvb_v[:, ncol * 8 : (ncol + 1) * 8, 0:HD],
                            psb[:].rearrange("p (h c) -> p h c", c=HD),
                        )
                    for so in range(SO):
                        nc.vector.tensor_add(VA[:, so, :], VA[:, so, :], bvb[:])

                if i >= 1:
                    # last tp of pair i-1 spills here; then ship the pair
                    tp_burst(1, 1, 3, opr)
                    ship_pair(i - 1, opr)

            # ---------------- epilogue: AV(7) + tp(7) ----------------
            ring = (NP - 1) % 2
            opr = (NP - 1) % 2
            pend = None
            for b, (half, qc) in enumerate(av_seq):
                av_burst(NP - 1, half, qc, ring, b)
                if pend is not None:
                    tp_burst(pend[0], pend[1], pend[2], opr)
                pend = (half, qc, b)
            tp_burst(pend[0], pend[1], pend[2], opr)
            ship_pair(NP - 1, opr)

    nc.compile()
    return nc


def _get_program(with_bv: bool):
    key = with_bv
    if key not in _programs:
        _programs[key] = _build_program(with_bv)
    return _programs[key]


def _in_maps(hidden_states, Wq, bq, Wk, bk, Wv, bv):
    wq = np.ascontiguousarray(Wq, np.float32)
    wk = np.ascontiguousarray(Wk, np.float32)
    wv = np.ascontiguousarray(Wv, np.float32)
    bq = np.ascontiguousarray(bq, np.float32)
    bk = np.ascontiguousarray(bk, np.float32)
    bv = np.ascontiguousarray(bv, np.float32)
    return [
        {
            "xt": np.ascontiguousarray(hidden_states[b].T, np.float32),
            "wq": wq, "wk": wk, "wv": wv, "bq": bq, "bk": bk, "bv": bv,
        }
        for b in range(B)
    ]


def kernel(hidden_states, Wq, bq, Wk, bk, Wv, bv):
    hidden_states = np.ascontiguousarray(hidden_states, dtype=np.float32)
    with_bv = bool(np.any(np.asarray(bv) != 0))
    nc = _get_program(with_bv)
    in_maps = _in_maps(hidden_states, Wq, bq, Wk, bk, Wv, bv)
    last_err = None
    for _attempt in range(3):
        try:
            res = run_bass_kernel_spmd(nc, in_maps, list(range(B)))
            return np.stack([res.results[b]["out"] for b in range(B)], axis=0)
        except Exception as e:  # transient NRT device errors recover on retry
            last_err = e
            import time
            time.sleep(3)
    raise last_err


# revision 5
# speedup vs baseline: 1.4224x; 1.4224x over previous
"""ConstituencyAwareAttention Trainium2 kernel (v2).

Strategy: pure data parallelism -- B=8 batch elements across 8 NeuronCores,
one full attention problem per core (S=1024, H=1024, nh=16, hd=64).

v2 restructure (vs v1 baseline at ~331us):
  * Chunked, prioritized input DMA: X chunks + first Q/K weight slices land
    first, so the bootstrap Q/K projection starts at ~10us instead of 40us.
    Remaining weight slices stream in under compute.
  * Software-pipelined pair slots: slot i emits scores(i)+exp(i) interleaved
    per-kt with proj(i+1) bursts and AV(i-1)+transpose(i-1) bursts, so the
    PE never drains while the scalar engine (exp, the secondary bottleneck
    at ~142us total) is continuously fed.
  * All matmul accumulation runs in single-bank bursts (8 chained MMs) so
    PSUM fits in exactly 8 banks: scores 4 (2 halves x [128,1024]) +
    proj ring 2 + work ring 2 (shared by V-proj / AV-ctx / PE-transposes).
  * exp is scalar-engine-only; penalty-fix multiplies moved to gpsimd;
    PSUM evacuations on vector -- keeps the exp stream stall-free.
  * Output shipped per head-pair from a small rotating buffer.
"""

import math
import sys

if "/opt/trn_rl_repo" not in sys.path:
    sys.path.insert(0, "/opt/trn_rl_repo")

import numpy as np

import concourse.bacc as bacc
import concourse.tile as tile
from concourse import mybir
from concourse.bass_utils import run_bass_kernel_spmd
from concourse.masks import make_identity

F16 = mybir.dt.float16
F32 = mybir.dt.float32

B, S, H = 8, 1024, 1024
NH, HD = 16, 64
P = 128
SO = S // P   # 8 S-chunks
KO = H // P   # 8 contraction chunks
NP = NH // 2  # 8 head pairs
PEN = 0.5
FIX = float(math.exp(PEN))
SCALE = 1.0 / math.sqrt(HD)

_programs = {}


def _build_program(with_bv: bool):
    nc = bacc.Bacc("TRN2", target_bir_lowering=False, debug=False)

    xt = nc.dram_tensor("xt", [H, S], F32, kind="ExternalInput").ap()
    wq = nc.dram_tensor("wq", [H, H], F32, kind="ExternalInput").ap()
    wk = nc.dram_tensor("wk", [H, H], F32, kind="ExternalInput").ap()
    wv = nc.dram_tensor("wv", [H, H], F32, kind="ExternalInput").ap()
    bq = nc.dram_tensor("bq", [H], F32, kind="ExternalInput").ap()
    bk = nc.dram_tensor("bk", [H], F32, kind="ExternalInput").ap()
    bv = nc.dram_tensor("bv", [H], F32, kind="ExternalInput").ap()
    out = nc.dram_tensor("out", [S, H], F32, kind="ExternalOutput").ap()

    Exp = mybir.ActivationFunctionType.Exp

    xt_r = xt.rearrange("(ho hp) s -> hp ho s", hp=P)      # [128, KO, S]
    wq_r = wq.rearrange("(kp_o kp) n -> kp kp_o n", kp=P)  # [128, KO, H]
    wk_r = wk.rearrange("(kp_o kp) n -> kp kp_o n", kp=P)
    wv_r = wv.rearrange("(kp_o kp) n -> kp kp_o n", kp=P)
    out_r = out.rearrange("(o p) n -> p o n", p=P)         # [128, SO, H]

    with tile.TileContext(nc) as tc:
        with (
            tc.tile_pool(name="persist", bufs=1) as persist,
            tc.tile_pool(name="stage", bufs=1) as stage,
            tc.tile_pool(name="probs", bufs=1) as probs,
            tc.tile_pool(name="score_ps", bufs=1, space="PSUM") as score_ps,
            tc.tile_pool(name="proj_ps", bufs=2, space="PSUM") as proj_ps,
            tc.tile_pool(name="work_ps", bufs=2, space="PSUM") as work_ps,
        ):
            # ---------------- persistent SBUF ----------------
            XT = persist.tile([P, KO, S], F16, name="XT")
            # Q^T/K^T live as 3-deep per-pair rings (each pair's slice is
            # produced one slot ahead of its scores)
            QT3 = persist.tile([P, 3, S], F16, name="QT3")
            KT3 = persist.tile([P, 3, S], F16, name="KT3")
            VA = persist.tile([P, SO, NH * (HD + 1)], F16, name="VA")
            wqh = persist.tile([P, KO, H], F16, name="wqh")
            wkh = persist.tile([P, KO, H], F16, name="wkh")
            wvh = persist.tile([P, KO, H], F16, name="wvh")
            ident = persist.tile([P, P], F32, name="ident")
            nbias = persist.tile([P, 1], F32, name="nbias")
            bq_s = persist.tile([P, KO], F32, name="bq_s")
            bk_s = persist.tile([P, KO], F32, name="bk_s")

            make_identity(nc, ident[:])
            nc.vector.memset(nbias[:], -PEN)
            # ones columns of V_aug only; V block copies fill the rest
            VA_v = VA[:].rearrange("p s (h c) -> p s h c", c=HD + 1)
            nc.vector.memset(VA_v[:, :, :, HD : HD + 1], 1.0)

            # ---------------- DMA: chunked, prioritized ----------------
            # Only 3 hw DMA queues: sync (SP), scalar (Activation), gpsimd.
            bq32 = stage.tile([P, KO], F32, name="bq32")
            bk32 = stage.tile([P, KO], F32, name="bk32")
            nc.sync.dma_start(out=bq32[:], in_=bq.rearrange("(o p) -> p o", p=P))
            nc.sync.dma_start(out=bk32[:], in_=bk.rearrange("(o p) -> p o", p=P))
            nc.vector.tensor_scalar_mul(bq_s[:], bq32[:], SCALE)
            nc.vector.tensor_copy(bk_s[:], bk32[:])

            # fp32 staging rings
            x32 = [
                stage.tile([P, S], F32, name=f"x32_{k}", tag="x32", bufs=2)
                for k in range(KO)
            ]
            wq32 = [None] * NP
            wk32 = [None] * NP
            wv32 = [None] * KO
            for mo in range(NP):
                wq32[mo] = stage.tile([P, KO, P], F32, name=f"wq32_{mo}", tag="w32", bufs=3)
                wk32[mo] = stage.tile([P, KO, P], F32, name=f"wk32_{mo}", tag="w32", bufs=3)
            for kh in range(KO):
                wv32[kh] = stage.tile([P, H], F32, name=f"wv32_{kh}", tag="wv32", bufs=2)

            # first q/k slices (bootstrap + slot-0 proj) ahead of X on sync;
            # X split across all 3 queues; Wv next; remaining q/k slices in
            # need order round-robin.
            nc.sync.dma_start(out=wq32[0][:], in_=wq_r[:, :, 0:P])
            nc.sync.dma_start(out=wk32[0][:], in_=wk_r[:, :, 0:P])
            for kh, eng in [(1, nc.scalar), (4, nc.scalar), (7, nc.scalar),
                            (2, nc.gpsimd), (5, nc.gpsimd),
                            (0, nc.sync), (3, nc.sync), (6, nc.sync)]:
                eng.dma_start(out=x32[kh][:], in_=xt_r[:, kh, :])
            nc.scalar.dma_start(out=wq32[1][:], in_=wq_r[:, :, P : 2 * P])
            nc.gpsimd.dma_start(out=wk32[1][:], in_=wk_r[:, :, P : 2 * P])
            for kh in range(KO):
                eng = nc.scalar if kh % 2 == 0 else nc.gpsimd
                eng.dma_start(out=wv32[kh][:], in_=wv_r[:, kh, :])
            rr = [nc.sync, nc.scalar, nc.gpsimd]
            idx = 0
            for mo in range(2, NP):
                rr[idx % 3].dma_start(out=wq32[mo][:], in_=wq_r[:, :, mo * P : (mo + 1) * P])
                idx += 1
                rr[idx % 3].dma_start(out=wk32[mo][:], in_=wk_r[:, :, mo * P : (mo + 1) * P])
                idx += 1

            wqh_v = wqh.rearrange("p o (m c) -> p o m c", c=P)
            wkh_v = wkh.rearrange("p o (m c) -> p o m c", c=P)

            # X casts on vector (gate everything)
            for kh in range(KO):
                nc.vector.tensor_copy(XT[:, kh, :], x32[kh][:])
            # first two q/k slice casts on vector (fast, before the exp storm)
            for mo in range(2):
                nc.vector.tensor_copy(wqh_v[:, :, mo, :], wq32[mo][:])
                nc.vector.tensor_copy(wkh_v[:, :, mo, :], wk32[mo][:])
            # Wv casts split vector/gpsimd (gate V bursts in slot 0)
            for kh in range(KO):
                nc.vector.tensor_copy(wvh[:, kh, :], wv32[kh][:])

            # probs rings: 2 pairs in flight (scored / being consumed by AV)
            prT = [
                [
                    probs.tile([P, KO, S], F16, name=f"prT_{h}_{r}")
                    for r in range(2)
                ]
                for h in range(2)
            ]
            ctxt_sb = [
                probs.tile([HD + 1, 512], F32, name=f"ctxt_sb{r}")
                for r in range(4)
            ]
            inv_sb = [
                probs.tile([P, 4], F32, name=f"inv{r}")
                for r in range(4)
            ]
            out_pair = [
                probs.tile([P, SO, P], F32, name=f"out_pair{r}")
                for r in range(2)
            ]

            # score psum: one [128, S] tile per half, reused across kt
            pst = [
                score_ps.tile([P, S], F32, name=f"pst{h}", tag=f"pst{h}")
                for h in range(2)
            ]

            # ---------------- emission helpers ----------------
            def cast_w_slice(mo):
                """fp16-cast q/k slice mo (JIT) on vector."""
                nc.vector.tensor_copy(wqh_v[:, :, mo, :], wq32[mo][:])
                nc.vector.tensor_copy(wkh_v[:, :, mo, :], wk32[mo][:])

            def proj_burst(mo, which, sc):
                """One Q or K projection burst: 8 chained MMs -> 1 psum bank,
                evacuated to the QT/KT ring with scale+bias fused."""
                wsb = wqh if which == "q" else wkh
                r = mo % 3
                ps = proj_ps.tile([P, 512], F32, name="ps", tag="proj")
                for kh in range(KO):
                    nc.tensor.matmul(
                        ps[:],
                        wsb[:, kh, mo * P : (mo + 1) * P],
                        XT[:, kh, sc * 512 : (sc + 1) * 512],
                        start=(kh == 0),
                        stop=(kh == KO - 1),
                    )
                if which == "q":
                    nc.vector.tensor_scalar(
                        QT3[:, r, sc * 512 : (sc + 1) * 512], ps[:],
                        SCALE, bq_s[:, mo : mo + 1],
                        mybir.AluOpType.mult, mybir.AluOpType.add,
                    )
                else:
                    nc.vector.tensor_scalar_add(
                        KT3[:, r, sc * 512 : (sc + 1) * 512], ps[:],
                        bk_s[:, mo : mo + 1],
                    )

            def v_burst(so, ncol):
                """V projection burst: 8 chained MMs -> work ring bank, then
                strided copy into V_aug (leaving the ones columns alone)."""
                ps = work_ps.tile([P, 512], F32, name="vps", tag="work")
                for kh in range(KO):
                    nc.tensor.matmul(
                        ps[:],
                        XT[:, kh, so * P : (so + 1) * P],
                        wvh[:, kh, ncol * 512 : (ncol + 1) * 512],
                        start=(kh == 0),
                        stop=(kh == KO - 1),
                    )
                va_v = VA[:, so, :].rearrange("p (h c) -> p h c", c=HD + 1)
                nc.vector.tensor_copy(
                    va_v[:, ncol * 8 : (ncol + 1) * 8, 0:HD],
                    ps[:].rearrange("p (h c) -> p h c", c=HD),
                )

            def score_kt(i, kt, ring):
                """scores + exp for (pair i, k-chunk kt); 4 row-packed MMs,
                2 scalar exps, gpsimd penalty-fix."""
                r = i % 3
                for half in range(2):
                    lo = half * 64
                    for qc in range(2):
                        nc.tensor.matmul(
                            pst[half][:, qc * 512 : (qc + 1) * 512],
                            KT3[lo : lo + 64, r, kt * P : (kt + 1) * P],
                            QT3[lo : lo + 64, r, qc * 512 : (qc + 1) * 512],
                            start=True,
                            stop=True,
                            tile_position=(lo, 0),
                        )
                for half in range(2):
                    dst = prT[half][ring]
                    nc.scalar.activation(
                        dst[:, kt, :], pst[half][:], Exp, bias=nbias[:]
                    )
                    nc.vector.tensor_scalar_mul(
                        dst[0:64, kt, kt * P : kt * P + 64],
                        dst[0:64, kt, kt * P : kt * P + 64],
                        FIX,
                    )
                    nc.vector.tensor_scalar_mul(
                        dst[64:128, kt, kt * P + 64 : (kt + 1) * P],
                        dst[64:128, kt, kt * P + 64 : (kt + 1) * P],
                        FIX,
                    )

            def av_burst(i, half, qc, ring, slot4):
                """AV for (pair i, head-half, q-chunk qc): 8 chained MMs into
                a work-ring bank (65 rows: 64 ctx dims + denominator), then
                fp32 evacuation to SBUF."""
                h = 2 * i + half
                ctx = work_ps.tile([P, 512], F32, name="ctx", tag="work")
                for kt in range(KO):
                    nc.tensor.matmul(
                        ctx[0 : HD + 1, :],
                        VA[:, kt, h * (HD + 1) : (h + 1) * (HD + 1)],
                        prT[half][ring][:, kt, qc * 512 : (qc + 1) * 512],
                        start=(kt == 0),
                        stop=(kt == KO - 1),
                    )
                nc.vector.tensor_copy(ctxt_sb[slot4][:], ctx[0 : HD + 1, :])

            def tp_burst(half, qc, slot4, opr):
                """Transpose ctx^T back to [q, d] + normalize into out_pair."""
                tp = work_ps.tile([P, 512], F32, name="tp", tag="work")
                tpv = tp[:, 0 : 4 * (HD + 1)].rearrange(
                    "p (c e) -> p c e", e=HD + 1
                )
                for c4 in range(4):
                    nc.tensor.transpose(
                        tpv[:, c4, :],
                        ctxt_sb[slot4][:, c4 * P : (c4 + 1) * P],
                        ident[0 : HD + 1, 0 : HD + 1],
                    )
                nc.vector.reciprocal(inv_sb[slot4][:], tpv[:, :, HD])
                for c4 in range(4):
                    so = qc * 4 + c4
                    nc.vector.tensor_scalar_mul(
                        out_pair[opr][:, so, half * HD : (half + 1) * HD],
                        tpv[:, c4, 0:HD],
                        inv_sb[slot4][:, c4 : c4 + 1],
                    )

            def ship_pair(i, opr):
                nc.sync.dma_start(
                    out=out_r[:, :, i * P : (i + 1) * P], in_=out_pair[opr][:]
                )

            # ---------------- bootstrap: proj(0) ----------------
            for sc in range(2):
                proj_burst(0, "q", sc)
            for sc in range(2):
                proj_burst(0, "k", sc)

            # ---------------- pair slots ----------------
            # slot i: scores(i) + exp(i) | proj(i+1) | slot0: V proj
            #         | i>=1: AV(i-1) + tp(i-1) + ship(i-1)
            av_seq = [(0, 0), (0, 1), (1, 0), (1, 1)]
            for i in range(NP):
                ring = i % 2
                pring = (i - 1) % 2
                opr = (i - 1) % 2

                for kt in range(KO):
                    score_kt(i, kt, ring)

                    # projection burst piece for pair i+1 (one per 2 steps)
                    if i + 1 < NP:
                        if kt == 0:
                            proj_burst(i + 1, "q", 0)
                        elif kt == 2:
                            proj_burst(i + 1, "q", 1)
                        elif kt == 4:
                            proj_burst(i + 1, "k", 0)
                        elif kt == 6:
                            proj_burst(i + 1, "k", 1)
                        elif kt == 7 and i + 2 < NP:
                            cast_w_slice(i + 2)

                    if i == 0:
                        # V projection fills the AV slots of slot 0
                        if kt < 4:
                            v_burst(2 * kt, 0)
                            v_burst(2 * kt, 1)
                        else:
                            v_burst(2 * (kt - 4) + 1, 0)
                            v_burst(2 * (kt - 4) + 1, 1)
                    else:
                        # AV(i-1) bursts on even steps, tp(i-1) on odd steps
                        if kt % 2 == 0:
                            half, qc = av_seq[kt // 2]
                            av_burst(i - 1, half, qc, pring, kt // 2)
                        elif kt >= 3:
                            half, qc = av_seq[(kt - 3) // 2]
                            tp_burst(half, qc, (kt - 3) // 2, opr)

                if i == 0 and with_bv:
                    # out += bv exactly (softmax rows sum to 1): broadcast bv
                    # across partitions via PE, add into V_aug.
                    ones1 = persist.tile([1, P], F16, name="ones1")
                    nc.vector.memset(ones1[:], 1.0)
                    bv1 = persist.tile([1, H], F16, name="bv1")
                    bv1_32 = persist.tile([1, H], F32, name="bv1_32")
                    nc.sync.dma_start(out=bv1_32[:], in_=bv[None, :])
                    nc.vector.tensor_copy(bv1[:], bv1_32[:])
                    bvb = persist.tile([P, NH * (HD + 1)], F16, name="bvb")
                    nc.vector.memset(bvb[:], 0.0)
                    bvb_v = bvb.rearrange("p (h c) -> p h c", c=HD + 1)
                    for ncol in range(2):
                        psb = proj_ps.tile([P, 512], F32, name="psb", tag="proj")
                        nc.tensor.matmul(
                            psb[:], ones1[:], bv1[:, ncol * 512 : (ncol + 1) * 512],
                            start=True, stop=True,
                        )
                        nc.vector.tensor_copy(
                            bvb_v[:, ncol * 8 : (ncol + 1) * 8, 0:HD],
                            psb[:].rearrange("p (h c) -> p h c", c=HD),
                        )
                    for so in range(SO):
                        nc.vector.tensor_add(VA[:, so, :], VA[:, so, :], bvb[:])

                if i >= 1:
                    # last tp of pair i-1 spills here; then ship the pair
                    tp_burst(1, 1, 3, opr)
                    ship_pair(i - 1, opr)

            # ---------------- epilogue: AV(7) + tp(7) ----------------
            ring = (NP - 1) % 2
            opr = (NP - 1) % 2
            pend = None
            for b, (half, qc) in enumerate(av_seq):
                av_burst(NP - 1, half, qc, ring, b)
                if pend is not None:
                    tp_burst(pend[0], pend[1], pend[2], opr)
                pend = (half, qc, b)
            tp_burst(pend[0], pend[1], pend[2], opr)
            ship_pair(NP - 1, opr)

    nc.compile()
    return nc


def _get_program(with_bv: bool):
    key = with_bv
    if key not in _programs:
        _programs[key] = _build_program(with_bv)
    return _programs[key]


def _in_maps(hidden_states, Wq, bq, Wk, bk, Wv, bv):
    wq = np.ascontiguousarray(Wq, np.float32)
    wk = np.ascontiguousarray(Wk, np.float32)
    wv = np.ascontiguousarray(Wv, np.float32)
    bq = np.ascontiguousarray(bq, np.float32)
    bk = np.ascontiguousarray(bk, np.float32)
    bv = np.ascontiguousarray(bv, np.float32)
    return [
        {
            "xt": np.ascontiguousarray(hidden_states[b].T, np.float32),
            "wq": wq, "wk": wk, "wv": wv, "bq": bq, "bk": bk, "bv": bv,
        }
        for b in range(B)
    ]


def kernel(hidden_states, Wq, bq, Wk, bk, Wv, bv):
    hidden_states = np.ascontiguousarray(hidden_states, dtype=np.float32)
    with_bv = bool(np.any(np.asarray(bv) != 0))
    nc = _get_program(with_bv)
    in_maps = _in_maps(hidden_states, Wq, bq, Wk, bk, Wv, bv)
    last_err = None
    for _attempt in range(3):
        try:
            res = run_bass_kernel_spmd(nc, in_maps, list(range(B)))
            return np.stack([res.results[b]["out"] for b in range(B)], axis=0)
        except Exception as e:  # transient NRT device errors recover on retry
            last_err = e
            import time
            time.sleep(3)
    raise last_err
